# revision 43
# baseline (speedup 1.0000x reference)
"""Block-circulant matmul kernel for 8 Trainium2 NeuronCores.

Reference op (per token row x of shape (4096,)):
    y = (x*d) @ M + bias,  M[(j,m),(i,n)] = W[i,j,(m-n)%256]  (circulant blocks)

Real-DFT factorization in three matmul stages per core, data-parallel over
batch (1024 tokens/core), software-pipelined over 2 token chunks of 512 so
stage-N compute of one chunk overlaps the corner-turn DMAs of the other:
  stage1: per input block j, project onto the 256-col real DFT basis
  stage2: per (slot-group gl, half pb), one 128x128 block-diag frequency mix
  stage3: per output block i, inverse real DFT basis + bias
Between stages, two SBUF->SBUF corner-turn DMA passes regroup the data
(frequency-major <-> block-major); each step is a plain 2-D slice DMA
([128,2048] <-> [8,16384], 4KB descriptors / 256KB per transfer, strided
dst/src partitions so each hits 8 distinct SDMA engines).  Queueing: x/y
ride sync (HWDGE, 1MB transfers, 8KB descriptors); turns ride gpsimd
(SWDGE, cheap issue) with 1/3 overflow on scalar; DVE and ACT do nothing
but PSUM evacuation (alternating; per-partition bias port for stage 3);
one 8-buffer single-bank PSUM pool keeps the PE ahead of evacuation.
Compute/IO in bf16 (PSUM accumulation fp32).  Measured ~162-164us on HW
(vs 235us baseline); DMA-engine-bound: SBUF->SBUF moves at ~10GB/s/engine
(vs ~26 for HBM), so the 16.8MB of corner turns set the floor.

Self-contained: shapes hardcoded; no sibling imports.
"""
import os
import sys

for _p in ("/root/.axon_site", "/root/.axon_site/_ro/trn_rl_repo", "/root/.axon_site/_ro/pypackages"):
    if _p not in sys.path:
        sys.path.append(_p)

import numpy as np
import ml_dtypes

import concourse.bass as bass
import concourse.tile as tile
from concourse import bacc, mybir
from concourse import bass_utils

N_CORES = 8
B = 8192
D = 4096
BS = 256
K = 16               # blocks per side
NSLOT = BS // 2      # 128 frequency pair-slots
NT = B // N_CORES    # tokens per core (1024)
NC = 2               # token chunks per core
TC = NT // NC        # tokens per chunk (512)

F32 = mybir.dt.float32
BF16 = mybir.dt.bfloat16
BF16_NP = ml_dtypes.bfloat16

LAST_EXEC_NS = None
_CACHE = {}

# r = 2q+c indexes (slot-in-group q, component c); device partition layouts:
#   u  partition p = 16*r + gl   (gl = slot-group within pb-half)
#   u2 partition p = 16*r + j    (j  = input block)
#   v2 partition p = 16*r + i    (i  = output block)
#   v  partition p = 16*r + gl
# Column layouts per chunk (bf16, TC=512 tokens):
#   u  [128, j*1024 + pb*512 + t]
#   u2 [128, gl*1024 + pb*512 + t]
#   v2 [128, gl*1024 + pb*512 + t]
#   v  [128, i*1024 + pb*512 + t]
# corner-turn 1 (per j):  u2[j::16, :] = u[:, j*1024:(j+1)*1024]
# corner-turn 2 (per i):  v[:, i*1024:(i+1)*1024] = v2[i::16, :]
# (flat element orders match: (r, gl|j, pb, t) on both sides)


# ---------------------------------------------------------------- host math

def _canonical_mats(W):
    m = np.arange(BS)
    T = np.zeros((BS, BS), np.float64)
    T[:, 0] = 1.0
    T[:, 1] = (-1.0) ** m
    for f in range(1, NSLOT):
        T[:, 2 * f] = np.cos(2 * np.pi * f * m / BS)
        T[:, 2 * f + 1] = np.sin(2 * np.pi * f * m / BS)

    Wf = np.fft.fft(W.astype(np.float64), axis=-1)
    p = Wf.real
    q = -Wf.imag

    jj = np.arange(K)
    M_slot = np.zeros((NSLOT, 2 * K, 2 * K), np.float64)
    for f in range(1, NSLOT):
        pf, qf = p[:, :, f], q[:, :, f]          # [i, j]
        M_slot[f][np.ix_(2 * jj, 2 * jj)] = pf.T
        M_slot[f][np.ix_(2 * jj + 1, 2 * jj)] = qf.T
        M_slot[f][np.ix_(2 * jj, 2 * jj + 1)] = qf.T
        M_slot[f][np.ix_(2 * jj + 1, 2 * jj + 1)] = -pf.T
    M_slot[0][np.ix_(2 * jj, 2 * jj)] = p[:, :, 0].T
    M_slot[0][np.ix_(2 * jj + 1, 2 * jj + 1)] = p[:, :, NSLOT].T

    n = np.arange(BS)
    R = np.zeros((BS, BS), np.float64)
    R[0, :] = 1.0 / BS
    R[1, :] = ((-1.0) ** n) / BS
    for f in range(1, NSLOT):
        R[2 * f, :] = 2.0 / BS * np.cos(2 * np.pi * f * n / BS)
        R[2 * f + 1, :] = -2.0 / BS * np.sin(2 * np.pi * f * n / BS)
    return T, M_slot, R


def _fft_host_mats(W, bias):
    T, M_slot, R = _canonical_mats(W)
    p_idx = np.arange(128)
    # partition p of u/v <-> gl = p%16, r = p//16, q = r//2, c = r%2
    gl_of = p_idx % 16
    q_of = (p_idx // 16) // 2
    c_of = (p_idx // 16) % 2

    # tb_dram (128, 4*128): [p_time, (mt*2+pb)*128+col] = T[mt*128+p_time, colmap(pb,col)]
    tb = np.zeros((128, 512), np.float64)
    for pb in range(2):
        slot = 64 * pb + 4 * gl_of + q_of
        cols = 2 * slot + c_of                    # canonical comp per device col
        for mt in range(2):
            tb[:, (mt * 2 + pb) * 128:(mt * 2 + pb + 1) * 128] = \
                T[mt * 128:(mt + 1) * 128, :][:, cols]

    # mix_dram (128, 32*128): [row, (gl*2+pb)*128+col]; rows/cols 16*(2q+c)+k
    # (strided partition layout 16r+j / 16r+i: corner turns hit 8 distinct
    #  DMA engines; consecutive layouts concentrate on 1-2 and are slower)
    mix = np.zeros((128, 32 * 128), np.float64)
    kk = np.arange(K)
    for gl in range(16):
        for pb in range(2):
            MG = np.zeros((128, 128), np.float64)
            for q in range(4):
                blk = M_slot[4 * (pb * 16 + gl) + q]
                for c in range(2):
                    for cp in range(2):
                        MG[np.ix_(16 * (2 * q + c) + kk, 16 * (2 * q + cp) + kk)] = \
                            blk[np.ix_(2 * kk + c, 2 * kk + cp)]
            g2 = gl * 2 + pb
            mix[:, g2 * 128:(g2 + 1) * 128] = MG

    # r_dram (128, 4*128): [p, (pb*2+nb)*128+col] = R[rowmap(pb,p), nb*128+col]
    # v partition p = 16*r + gl  (r = 2q+c)
    rd = np.zeros((128, 512), np.float64)
    for pb in range(2):
        rows = 2 * (64 * pb + 4 * gl_of + q_of) + c_of
        for nb in range(2):
            rd[:, (pb * 2 + nb) * 128:(pb * 2 + nb + 1) * 128] = \
                R[rows, :][:, nb * 128:(nb + 1) * 128]

    # bias (128, 32) f32: [p, i*2+nb] = bias[i*256 + nb*128 + p]
    bd = bias.astype(np.float64).reshape(K, 2, 128).transpose(2, 0, 1).reshape(128, 32)
    return (tb.astype(BF16_NP), mix.astype(BF16_NP), rd.astype(BF16_NP),
            np.ascontiguousarray(bd).astype(np.float32))


# ---------------------------------------------------------------- fft kernel

def _build_fft_nc():
    nc = bacc.Bacc("TRN2", target_bir_lowering=False, debug=False)
    # x_dev partition-major: [128, c*16384 + j*2048 + mt*1024... chunk-major:
    #   [128, c*(K*2*TC) + j*(2*TC) + mt*TC + t]  (4KB runs per x-tile)
    xT = nc.dram_tensor("xT", [128, K * 2 * NT], BF16, kind="ExternalInput").ap()
    tb_d = nc.dram_tensor("tb", [128, 512], BF16, kind="ExternalInput").ap()
    mix_d = nc.dram_tensor("mix", [128, 32 * 128], BF16, kind="ExternalInput").ap()
    r_d = nc.dram_tensor("rmat", [128, 512], BF16, kind="ExternalInput").ap()
    bias_d = nc.dram_tensor("biasd", [128, 32], F32, kind="ExternalInput").ap()
    # y_dev partition-major: [128, c*(32*TC) + ob*TC + t]
    yT = nc.dram_tensor("yT", [128, 32 * NT], BF16, kind="ExternalOutput").ap()

    CW = K * 2 * TC      # x cols per chunk (16384)
    ec = [0]

    def evac(dst, src, bias_ap=None):
        # alternate PSUM->SBUF evacuation between DVE and ACT
        if ec[0] % 2 == 0:
            if bias_ap is None:
                nc.vector.tensor_copy(dst, src)
            else:
                nc.vector.tensor_scalar_add(dst, src, bias_ap)
        else:
            if bias_ap is None:
                nc.scalar.copy(dst, src)
            else:
                nc.scalar.add(dst, src, bias_ap)
        ec[0] += 1

    with tile.TileContext(nc) as tc:
        with (
            tc.tile_pool(name="consts", bufs=1) as consts,
            tc.tile_pool(name="xpool", bufs=6) as xpool,
            tc.tile_pool(name="upool", bufs=2) as upool,      # u[c] then v2[c]
            tc.tile_pool(name="u2pool", bufs=2) as u2pool,    # u2[c] then v[c]
            tc.tile_pool(name="ypool", bufs=2) as ypool,
            tc.tile_pool(name="ps", bufs=8, space="PSUM") as psp,
        ):
            # tb on sync FIRST (scalar's ACT-table preamble delays its queue by
            # ~3us and tb gates the very first matmul); other consts on scalar.
            tb_sb = consts.tile([128, 512], BF16)
            nc.sync.dma_start(tb_sb[:], tb_d[:])
            mix_sb = consts.tile([128, 32 * 128], BF16)
            nc.scalar.dma_start(mix_sb[:], mix_d[:])
            r_sb = consts.tile([128, 512], BF16)
            nc.scalar.dma_start(r_sb[:], r_d[:])
            bias_sb = consts.tile([128, 32], F32)
            nc.scalar.dma_start(bias_sb[:], bias_d[:])

            # x tiles of 4 blocks (8KB/partition descriptors -> full HBM rate)
            x_t = {}
            for c in range(NC):
                for g in range(K // 4):
                    xt = xpool.tile([128, 8 * TC], BF16, tag="x", name=f"x_{c}_{g}")
                    nc.sync.dma_start(
                        xt[:], xT[:, c * CW + g * 8 * TC:c * CW + (g + 1) * 8 * TC])
                    x_t[(c, g)] = xt

            u_sb, u2_sb, v2_sb, v_sb = {}, {}, {}, {}
            for c in range(NC):
                u_sb[c] = upool.tile([128, CW], BF16, tag="uv2", name=f"u_{c}")
                u2_sb[c] = u2pool.tile([128, CW], BF16, tag="u2v", name=f"u2_{c}")

            # ---- stage 1 + corner-turn 1, per chunk, pipelined per block j ----
            for c in range(NC):
                for j in range(K):
                    for pb in range(2):
                        ps1 = psp.tile([128, TC], F32, tag="ps", name=f"ps1_{c}_{j}_{pb}")
                        for mt in range(2):
                            nc.tensor.matmul(
                                ps1[:],
                                tb_sb[:, (mt * 2 + pb) * 128:(mt * 2 + pb + 1) * 128],
                                x_t[(c, j // 4)][:, (j % 4) * 2 * TC + mt * TC:
                                                 (j % 4) * 2 * TC + (mt + 1) * TC],
                                start=(mt == 0), stop=(mt == 1),
                            )
                        evac(u_sb[c][:, j * 2 * TC + pb * TC:j * 2 * TC + (pb + 1) * TC],
                             ps1[:])
                    # corner-turn 1(c,j): u2[16r+j, (gl,pb,t)] = u[16r+gl, (j,pb,t)]
                    # deadline-balanced: chunk 0 (needed first, at s1(c1) end)
                    # all on the fast gpsimd queue; chunk 1 (needed one phase
                    # later) splits half onto scalar's slower 8-engine queue.
                    _e1 = nc.gpsimd if c == 0 else (nc.gpsimd, nc.scalar)[j % 2]
                    _e1.dma_start(
                        u2_sb[c][j::16, :],
                        u_sb[c][:, j * 2 * TC:(j + 1) * 2 * TC],
                    )

            # ---- stage 2, per chunk, per (slot-group gl, half pb) ----
            for c in range(NC):
                v2_sb[c] = upool.tile([128, CW], BF16, tag="uv2", name=f"v2_{c}")
                for gl in range(16):
                    for pb in range(2):
                        ps2 = psp.tile([128, TC], F32, tag="ps", name=f"ps2_{c}_{gl}_{pb}")
                        g2 = gl * 2 + pb
                        nc.tensor.matmul(
                            ps2[:],
                            mix_sb[:, g2 * 128:(g2 + 1) * 128],
                            u2_sb[c][:, gl * 2 * TC + pb * TC:
                                     gl * 2 * TC + (pb + 1) * TC],
                            start=True, stop=True,
                        )
                        evac(v2_sb[c][:, gl * 2 * TC + pb * TC:
                                      gl * 2 * TC + (pb + 1) * TC], ps2[:])

            # ---- corner-turn 2 + stage 3 + bias, per chunk, per block i ----
            for c in range(NC):
                v_sb[c] = u2pool.tile([128, CW], BF16, tag="u2v", name=f"v_{c}")
                y_t = None
                for i in range(K):
                    # corner-turn 2(c,i): v[16r+gl, (i,pb,t)] = v2[16r+i, (gl,pb,t)]
                    # same deadline balance: chunk 0 mostly gpsimd, chunk 1 half scalar
                    if c == 0:
                        _e2 = nc.scalar if i % 4 == 3 else nc.gpsimd
                    else:
                        _e2 = (nc.gpsimd, nc.scalar)[i % 2]
                    _e2.dma_start(
                        v_sb[c][:, i * 2 * TC:(i + 1) * 2 * TC],
                        v2_sb[c][i::16, :],
                    )
                    for nb in range(2):
                        ob = i * 2 + nb
                        if ob % 8 == 0:
                            y_t = ypool.tile([128, 8 * TC], BF16, tag="y",
                                             name=f"y_{c}_{ob // 8}")
                        ps3 = psp.tile([128, TC], F32, tag="ps", name=f"ps3_{c}_{ob}")
                        for pb in range(2):
                            nc.tensor.matmul(
                                ps3[:],
                                r_sb[:, (pb * 2 + nb) * 128:(pb * 2 + nb + 1) * 128],
                                v_sb[c][:, i * 2 * TC + pb * TC:
                                        i * 2 * TC + (pb + 1) * TC],
                                start=(pb == 0), stop=(pb == 1),
                            )
                        evac(y_t[:, (ob % 8) * TC:(ob % 8 + 1) * TC], ps3[:],
                             bias_sb[:, ob:ob + 1])
                        if ob % 8 == 7:
                            ig = ob // 8
                            nc.sync.dma_start(
                                yT[:, c * 32 * TC + ig * 8 * TC:
                                   c * 32 * TC + (ig + 1) * 8 * TC], y_t[:])
    nc.compile()
    return nc


# ---------------------------------------------------------------- entry point

def _run(nc, in_maps):
    global LAST_EXEC_NS
    trace = bool(os.environ.get("BASS_TRACE"))
    res = bass_utils.run_bass_kernel_spmd(
        nc, in_maps, list(range(N_CORES)), trace=trace,
        tmpdir=os.environ.get("BASS_TRACE_DIR") or None,
    )
    LAST_EXEC_NS = res.exec_time_ns
    return res


def kernel(x, W, d_bernoulli, bias):
    x = np.asarray(x, dtype=np.float32)
    W = np.asarray(W, dtype=np.float32)
    d_bernoulli = np.asarray(d_bernoulli, dtype=np.float32)
    bias = np.asarray(bias, dtype=np.float32)

    xT = np.ascontiguousarray((x * d_bernoulli[None, :]).T)

    if "fft" not in _CACHE:
        _CACHE["fft"] = _build_fft_nc()
    tb, mix, rd, bd = _fft_host_mats(W, bias)
    in_maps = []
    for c in range(N_CORES):
        xs = xT[:, c * NT:(c + 1) * NT]                    # (D, NT)
        # device layout [p, ch*16384 + j*2048 + mt*1024 + t]  (chunk-major)
        xd = (xs.reshape(K, 2, 128, NC, TC)
              .transpose(2, 3, 0, 1, 4)
              .reshape(128, K * 2 * NT))
        in_maps.append({
            "xT": np.ascontiguousarray(xd).astype(BF16_NP),
            "tb": tb, "mix": mix, "rmat": rd, "biasd": bd,
        })
    res = _run(_CACHE["fft"], in_maps)

    out = np.empty((B, D), dtype=np.float32)
    for c in range(N_CORES):
        # yT cols = ch*(32*TC) + ob*TC + t
        yv = np.asarray(res.results[c]["yT"]).reshape(128, NC, 32, TC)
        for ch in range(NC):
            blk = yv[:, ch].transpose(2, 1, 0).reshape(TC, D)   # [t, ob*128+p]
            out[c * NT + ch * TC:c * NT + (ch + 1) * TC, :] = blk.astype(np.float32)
    return out


# revision 44
# speedup vs baseline: 1.0721x; 1.0721x over previous
"""Block-circulant matmul kernel for 8 Trainium2 NeuronCores.

Reference op (per token row x of shape (4096,)):
    y = (x*d) @ M + bias,  M[(j,m),(i,n)] = W[i,j,(m-n)%256]  (circulant blocks)

Real-DFT factorization in three matmul stages per core, data-parallel over
batch (1024 tokens/core), software-pipelined over 2 token chunks of 512 so
stage-N compute of one chunk overlaps the corner-turn DMAs of the other:
  stage1: per input block j, project onto the 256-col real DFT basis
  stage2: per (slot-group gl, half pb), one 128x128 block-diag frequency mix
  stage3: per output block i, inverse real DFT basis + bias
Between stages, two SBUF->SBUF corner-turn DMA passes regroup the data
(frequency-major <-> block-major); each step is a plain 2-D slice DMA
([128,2048] <-> [8,16384], 4KB descriptors / 256KB per transfer, strided
dst/src partitions so each hits 8 distinct SDMA engines).  Queueing: x/y
ride sync (HWDGE, 1MB transfers, 8KB descriptors); turns ride gpsimd
(SWDGE, cheap issue) with 1/3 overflow on scalar; DVE and ACT do nothing
but PSUM evacuation (alternating; per-partition bias port for stage 3);
one 8-buffer single-bank PSUM pool keeps the PE ahead of evacuation.
Compute/IO in bf16 (PSUM accumulation fp32).  Measured ~162-164us on HW
(vs 235us baseline); DMA-engine-bound: SBUF->SBUF moves at ~10GB/s/engine
(vs ~26 for HBM), so the 16.8MB of corner turns set the floor.

Self-contained: shapes hardcoded; no sibling imports.
"""
import os
import sys

for _p in ("/root/.axon_site", "/root/.axon_site/_ro/trn_rl_repo", "/root/.axon_site/_ro/pypackages"):
    if _p not in sys.path:
        sys.path.append(_p)

import numpy as np
import ml_dtypes

import concourse.bass as bass
import concourse.tile as tile
from concourse import bacc, mybir
from concourse import bass_utils

N_CORES = 8
B = 8192
D = 4096
BS = 256
K = 16               # blocks per side
NSLOT = BS // 2      # 128 frequency pair-slots
NT = B // N_CORES    # tokens per core (1024)
NC = 2               # token chunks per core
TC = NT // NC        # tokens per chunk (512)

F32 = mybir.dt.float32
BF16 = mybir.dt.bfloat16
BF16_NP = ml_dtypes.bfloat16

LAST_EXEC_NS = None
_CACHE = {}

# r = 2q+c indexes (slot-in-group q, component c); device partition layouts:
#   u  partition p = 16*r + gl   (gl = slot-group within pb-half)
#   u2 partition p = 16*r + j    (j  = input block)
#   v2 partition p = 16*r + i    (i  = output block)
#   v  partition p = 16*r + gl
# Column layouts per chunk (bf16, TC=512 tokens):
#   u  [128, j*1024 + pb*512 + t]
#   u2 [128, gl*1024 + pb*512 + t]
#   v2 [128, gl*1024 + pb*512 + t]
#   v  [128, i*1024 + pb*512 + t]
# corner-turn 1 (per j):  u2[j::16, :] = u[:, j*1024:(j+1)*1024]
# corner-turn 2 (per i):  v[:, i*1024:(i+1)*1024] = v2[i::16, :]
# (flat element orders match: (r, gl|j, pb, t) on both sides)


# ---------------------------------------------------------------- host math

def _canonical_mats(W):
    m = np.arange(BS)
    T = np.zeros((BS, BS), np.float64)
    T[:, 0] = 1.0
    T[:, 1] = (-1.0) ** m
    for f in range(1, NSLOT):
        T[:, 2 * f] = np.cos(2 * np.pi * f * m / BS)
        T[:, 2 * f + 1] = np.sin(2 * np.pi * f * m / BS)

    Wf = np.fft.fft(W.astype(np.float64), axis=-1)
    p = Wf.real
    q = -Wf.imag

    jj = np.arange(K)
    M_slot = np.zeros((NSLOT, 2 * K, 2 * K), np.float64)
    for f in range(1, NSLOT):
        pf, qf = p[:, :, f], q[:, :, f]          # [i, j]
        M_slot[f][np.ix_(2 * jj, 2 * jj)] = pf.T
        M_slot[f][np.ix_(2 * jj + 1, 2 * jj)] = qf.T
        M_slot[f][np.ix_(2 * jj, 2 * jj + 1)] = qf.T
        M_slot[f][np.ix_(2 * jj + 1, 2 * jj + 1)] = -pf.T
    M_slot[0][np.ix_(2 * jj, 2 * jj)] = p[:, :, 0].T
    M_slot[0][np.ix_(2 * jj + 1, 2 * jj + 1)] = p[:, :, NSLOT].T

    n = np.arange(BS)
    R = np.zeros((BS, BS), np.float64)
    R[0, :] = 1.0 / BS
    R[1, :] = ((-1.0) ** n) / BS
    for f in range(1, NSLOT):
        R[2 * f, :] = 2.0 / BS * np.cos(2 * np.pi * f * n / BS)
        R[2 * f + 1, :] = -2.0 / BS * np.sin(2 * np.pi * f * n / BS)
    return T, M_slot, R


def _fft_host_mats(W, bias):
    T, M_slot, R = _canonical_mats(W)
    p_idx = np.arange(128)
    # partition p of u/v <-> gl = p%16, r = p//16, q = r//2, c = r%2
    gl_of = p_idx % 16
    q_of = (p_idx // 16) // 2
    c_of = (p_idx // 16) % 2

    # tb_dram (128, 4*128): [p_time, (mt*2+pb)*128+col] = T[mt*128+p_time, colmap(pb,col)]
    tb = np.zeros((128, 512), np.float64)
    for pb in range(2):
        slot = 64 * pb + 4 * gl_of + q_of
        cols = 2 * slot + c_of                    # canonical comp per device col
        for mt in range(2):
            tb[:, (mt * 2 + pb) * 128:(mt * 2 + pb + 1) * 128] = \
                T[mt * 128:(mt + 1) * 128, :][:, cols]

    # mix_dram (128, 32*128): [row, (gl*2+pb)*128+col]; rows/cols 16*(2q+c)+k
    # (strided partition layout 16r+j / 16r+i: corner turns hit 8 distinct
    #  DMA engines; consecutive layouts concentrate on 1-2 and are slower)
    mix = np.zeros((128, 32 * 128), np.float64)
    kk = np.arange(K)
    for gl in range(16):
        for pb in range(2):
            MG = np.zeros((128, 128), np.float64)
            for q in range(4):
                blk = M_slot[4 * (pb * 16 + gl) + q]
                for c in range(2):
                    for cp in range(2):
                        MG[np.ix_(16 * (2 * q + c) + kk, 16 * (2 * q + cp) + kk)] = \
                            blk[np.ix_(2 * kk + c, 2 * kk + cp)]
            g2 = gl * 2 + pb
            mix[:, g2 * 128:(g2 + 1) * 128] = MG

    # r_dram (128, 4*128): [p, (pb*2+nb)*128+col] = R[rowmap(pb,p), nb*128+col]
    # v partition p = 16*r + gl  (r = 2q+c)
    rd = np.zeros((128, 512), np.float64)
    for pb in range(2):
        rows = 2 * (64 * pb + 4 * gl_of + q_of) + c_of
        for nb in range(2):
            rd[:, (pb * 2 + nb) * 128:(pb * 2 + nb + 1) * 128] = \
                R[rows, :][:, nb * 128:(nb + 1) * 128]

    # bias (128, 32) f32: [p, i*2+nb] = bias[i*256 + nb*128 + p]
    bd = bias.astype(np.float64).reshape(K, 2, 128).transpose(2, 0, 1).reshape(128, 32)
    return (tb.astype(BF16_NP), mix.astype(BF16_NP), rd.astype(BF16_NP),
            np.ascontiguousarray(bd).astype(np.float32))


# ---------------------------------------------------------------- fft kernel

def _build_fft_nc():
    nc = bacc.Bacc("TRN2", target_bir_lowering=False, debug=False)
    # x_dev partition-major: [128, c*16384 + j*2048 + mt*1024... chunk-major:
    #   [128, c*(K*2*TC) + j*(2*TC) + mt*TC + t]  (4KB runs per x-tile)
    xT = nc.dram_tensor("xT", [128, K * 2 * NT], BF16, kind="ExternalInput").ap()
    tb_d = nc.dram_tensor("tb", [128, 512], BF16, kind="ExternalInput").ap()
    mix_d = nc.dram_tensor("mix", [128, 32 * 128], BF16, kind="ExternalInput").ap()
    r_d = nc.dram_tensor("rmat", [128, 512], BF16, kind="ExternalInput").ap()
    bias_d = nc.dram_tensor("biasd", [128, 32], F32, kind="ExternalInput").ap()
    # y_dev partition-major: [128, c*(32*TC) + ob*TC + t]
    yT = nc.dram_tensor("yT", [128, 32 * NT], BF16, kind="ExternalOutput").ap()

    CW = K * 2 * TC      # x cols per chunk (16384)
    ec = [0]

    def evac(dst, src, bias_ap=None):
        # alternate PSUM->SBUF evacuation between DVE and ACT
        if ec[0] % 2 == 0:
            if bias_ap is None:
                nc.vector.tensor_copy(dst, src)
            else:
                nc.vector.tensor_scalar_add(dst, src, bias_ap)
        else:
            if bias_ap is None:
                nc.scalar.copy(dst, src)
            else:
                nc.scalar.add(dst, src, bias_ap)
        ec[0] += 1

    with tile.TileContext(nc) as tc:
        with (
            tc.tile_pool(name="consts", bufs=1) as consts,
            tc.tile_pool(name="xpool", bufs=6) as xpool,
            tc.tile_pool(name="upool", bufs=2) as upool,      # u[c] then v2[c]
            tc.tile_pool(name="u2pool", bufs=2) as u2pool,    # u2[c] then v[c]
            tc.tile_pool(name="ypool", bufs=2) as ypool,
            tc.tile_pool(name="ps", bufs=8, space="PSUM") as psp,
        ):
            # tb on sync FIRST (scalar's ACT-table preamble delays its queue
            # ~3us and tb gates the first matmul); other consts on scalar.
            tb_sb = consts.tile([128, 512], BF16)
            nc.sync.dma_start(tb_sb[:], tb_d[:])
            mix_sb = consts.tile([128, 32 * 128], BF16)
            nc.scalar.dma_start(mix_sb[:], mix_d[:])
            r_sb = consts.tile([128, 512], BF16)
            nc.scalar.dma_start(r_sb[:], r_d[:])
            bias_sb = consts.tile([128, 32], F32)
            nc.scalar.dma_start(bias_sb[:], bias_d[:])

            # x tiles of 4 blocks (8KB/partition descriptors -> full HBM rate)
            x_t = {}
            for c in range(NC):
                for g in range(K // 4):
                    xt = xpool.tile([128, 8 * TC], BF16, tag="x", name=f"x_{c}_{g}")
                    nc.sync.dma_start(
                        xt[:], xT[:, c * CW + g * 8 * TC:c * CW + (g + 1) * 8 * TC])
                    x_t[(c, g)] = xt

            u_sb, u2_sb, v2_sb, v_sb = {}, {}, {}, {}
            for c in range(NC):
                u_sb[c] = upool.tile([128, CW], BF16, tag="uv2", name=f"u_{c}")
                u2_sb[c] = u2pool.tile([128, CW], BF16, tag="u2v", name=f"u2_{c}")

            # ---- stage 1 + corner-turn 1, per chunk, pipelined per block j ----
            for c in range(NC):
                for j in range(K):
                    for pb in range(2):
                        ps1 = psp.tile([128, TC], F32, tag="ps", name=f"ps1_{c}_{j}_{pb}")
                        for mt in range(2):
                            nc.tensor.matmul(
                                ps1[:],
                                tb_sb[:, (mt * 2 + pb) * 128:(mt * 2 + pb + 1) * 128],
                                x_t[(c, j // 4)][:, (j % 4) * 2 * TC + mt * TC:
                                                 (j % 4) * 2 * TC + (mt + 1) * TC],
                                start=(mt == 0), stop=(mt == 1),
                            )
                        evac(u_sb[c][:, j * 2 * TC + pb * TC:j * 2 * TC + (pb + 1) * TC],
                             ps1[:])
                    # corner-turn 1(c,j): u2[16r+j, (gl,pb,t)] = u[16r+gl, (j,pb,t)]
                    # 3/4 on gpsimd (16-engine SWDGE), 1/4 on scalar (8-engine);
                    # keeping sync free for x/y measurably beats 3-way spreading.
                    _e1 = nc.scalar if j % 3 == 2 else nc.gpsimd
                    _e1.dma_start(
                        u2_sb[c][j::16, :],
                        u_sb[c][:, j * 2 * TC:(j + 1) * 2 * TC],
                    )

            # ---- stage 2, per chunk, per (slot-group gl, half pb) ----
            for c in range(NC):
                v2_sb[c] = upool.tile([128, CW], BF16, tag="uv2", name=f"v2_{c}")
                for gl in range(16):
                    for pb in range(2):
                        ps2 = psp.tile([128, TC], F32, tag="ps", name=f"ps2_{c}_{gl}_{pb}")
                        g2 = gl * 2 + pb
                        nc.tensor.matmul(
                            ps2[:],
                            mix_sb[:, g2 * 128:(g2 + 1) * 128],
                            u2_sb[c][:, gl * 2 * TC + pb * TC:
                                     gl * 2 * TC + (pb + 1) * TC],
                            start=True, stop=True,
                        )
                        evac(v2_sb[c][:, gl * 2 * TC + pb * TC:
                                      gl * 2 * TC + (pb + 1) * TC], ps2[:])

            # ---- corner-turn 2 + stage 3 + bias, per chunk, per block i ----
            for c in range(NC):
                v_sb[c] = u2pool.tile([128, CW], BF16, tag="u2v", name=f"v_{c}")
                y_t = None
                for i in range(K):
                    # corner-turn 2(c,i): v[16r+gl, (i,pb,t)] = v2[16r+i, (gl,pb,t)]
                    _e2 = nc.scalar if i % 3 == 2 else nc.gpsimd
                    _e2.dma_start(
                        v_sb[c][:, i * 2 * TC:(i + 1) * 2 * TC],
                        v2_sb[c][i::16, :],
                    )
                    for nb in range(2):
                        ob = i * 2 + nb
                        if ob % 8 == 0:
                            y_t = ypool.tile([128, 8 * TC], BF16, tag="y",
                                             name=f"y_{c}_{ob // 8}")
                        ps3 = psp.tile([128, TC], F32, tag="ps", name=f"ps3_{c}_{ob}")
                        for pb in range(2):
                            nc.tensor.matmul(
                                ps3[:],
                                r_sb[:, (pb * 2 + nb) * 128:(pb * 2 + nb + 1) * 128],
                                v_sb[c][:, i * 2 * TC + pb * TC:
                                        i * 2 * TC + (pb + 1) * TC],
                                start=(pb == 0), stop=(pb == 1),
                            )
                        evac(y_t[:, (ob % 8) * TC:(ob % 8 + 1) * TC], ps3[:],
                             bias_sb[:, ob:ob + 1])
                        if ob % 8 == 7:
                            ig = ob // 8
                            nc.sync.dma_start(
                                yT[:, c * 32 * TC + ig * 8 * TC:
                                   c * 32 * TC + (ig + 1) * 8 * TC], y_t[:])
    nc.compile()
    return nc


# ---------------------------------------------------------------- entry point

def _run(nc, in_maps):
    global LAST_EXEC_NS
    trace = bool(os.environ.get("BASS_TRACE"))
    res = bass_utils.run_bass_kernel_spmd(
        nc, in_maps, list(range(N_CORES)), trace=trace,
        tmpdir=os.environ.get("BASS_TRACE_DIR") or None,
    )
    LAST_EXEC_NS = res.exec_time_ns
    return res


def kernel(x, W, d_bernoulli, bias):
    x = np.asarray(x, dtype=np.float32)
    W = np.asarray(W, dtype=np.float32)
    d_bernoulli = np.asarray(d_bernoulli, dtype=np.float32)
    bias = np.asarray(bias, dtype=np.float32)

    xT = np.ascontiguousarray((x * d_bernoulli[None, :]).T)

    if "fft" not in _CACHE:
        _CACHE["fft"] = _build_fft_nc()
    tb, mix, rd, bd = _fft_host_mats(W, bias)
    in_maps = []
    for c in range(N_CORES):
        xs = xT[:, c * NT:(c + 1) * NT]                    # (D, NT)
        # device layout [p, ch*16384 + j*2048 + mt*1024 + t]  (chunk-major)
        xd = (xs.reshape(K, 2, 128, NC, TC)
              .transpose(2, 3, 0, 1, 4)
              .reshape(128, K * 2 * NT))
        in_maps.append({
            "xT": np.ascontiguousarray(xd).astype(BF16_NP),
            "tb": tb, "mix": mix, "rmat": rd, "biasd": bd,
        })
    res = _run(_CACHE["fft"], in_maps)

    out = np.empty((B, D), dtype=np.float32)
    for c in range(N_CORES):
        # yT cols = ch*(32*TC) + ob*TC + t
        yv = np.asarray(res.results[c]["yT"]).reshape(128, NC, 32, TC)
        for ch in range(NC):
            blk = yv[:, ch].transpose(2, 1, 0).reshape(TC, D)   # [t, ob*128+p]
            out[c * NT + ch * TC:c * NT + (ch + 1) * TC, :] = blk.astype(np.float32)
    return out
